# revision 12
# baseline (speedup 1.0000x reference)
import math
import sys

import numpy as np

if "/opt/trn_rl_repo" not in sys.path:
    sys.path.insert(0, "/opt/trn_rl_repo")

import ml_dtypes

BF16 = ml_dtypes.bfloat16

B, L, H, N2, NB = 16, 1024, 256, 64, 6
STEP_EMB, NFEAT = 128, 4
NCORES = 8
BLOC = B // NCORES  # 2 batch elements per core
P = 128
LT = L // P          # 8 l-tiles
HT = H // P          # 2 h-tiles
BH = BLOC * H        # 512 bh columns in zT layout
NBT = 2 * L // P     # 16 packed-bin tiles (Re 0..7, Im 8..15)

_LAST_EXEC_NS = None
_BUILT = None  # cached (nc, meta)


# ---------------------------------------------------------------------------
# host-side preparation (weights only; everything O(H^2) or O(H*N2*L) once)
# ---------------------------------------------------------------------------

def _silu(x):
    return x / (1.0 + np.exp(-x))


def _dft_mats():
    lk = np.arange(L, dtype=np.float64)
    ang = (2.0 * np.pi / (2 * L)) * np.outer(lk, lk)  # (l, k)
    fre = np.cos(ang, dtype=np.float64).astype(BF16)
    fim = (-np.sin(ang)).astype(BF16)
    ck = np.full(L, 2.0 / (2 * L)); ck[0] = 1.0 / (2 * L)
    gre = (ck[:, None] * np.cos(ang)).astype(BF16)        # (k, t) (ang symmetric)
    gim = (-2.0 / (2 * L) * np.sin(ang)).astype(BF16)     # (k, t)
    gt = np.concatenate([gre, gim], 0)                    # (2048, 1024)
    return fre, fim, gt


def _khat(inp):
    """Per-block rfft of the bidirectional S4D kernel, ln_g folded in.
    Returns (NB, 8, 2, 128, 256) f32: [block, mt, re/im, bin-in-tile, h]."""
    out = np.empty((NB, NBT // 2, 2, P, H), np.float32)
    dfold = np.empty((NB, H), np.float32)
    for i in range(NB):
        dt = np.exp(inp["log_dt"][i].astype(np.float64))
        A = -inp["A_re"][i].astype(np.float64) + 1j * inp["A_im"][i].astype(np.float64)
        dtA = (dt[:, None] * A).astype(np.complex64)          # (H,N2)
        C = (inp["C_re"][i] + 1j * inp["C_im"][i]).astype(np.complex64)
        Bt = C * (np.exp(dtA) - 1.0) / dtA * dt[:, None].astype(np.complex64)
        r = np.exp(dtA)
        V = np.ones((H, N2, 1), np.complex64)
        p = r.copy()
        while V.shape[-1] < L:
            V = np.concatenate([V, V * p[:, :, None]], -1)
            p = p * p
        # K[c,h,l] = 2*Re( Bt[c,h,:] @ V[h,:,:] )
        K = 2.0 * np.real(np.matmul(Bt.transpose(1, 0, 2), V))  # (H,2,L)
        k_full = np.empty((H, 2 * L), np.float32)
        k_full[:, :L] = K[:, 0]
        k_full[:, L:] = K[:, 1, ::-1]
        Kh = np.fft.rfft(k_full, axis=-1)[:, :L]  # (H, 1024), Nyquist dropped
        g = inp["ln_g"][i][:, None]  # (H,1)
        re = (Kh.real * g).astype(np.float32).T  # (1024 bins, H)
        im = (Kh.imag * g).astype(np.float32).T
        out[i, :, 0] = re.reshape(NBT // 2, P, H)
        out[i, :, 1] = im.reshape(NBT // 2, P, H)
        dfold[i] = inp["D"][i] * inp["ln_g"][i]
    return out, dfold


def _host_prep(inp):
    fre, fim, gt = _dft_mats()
    khat, dfold = _khat(inp)

    half = STEP_EMB // 2
    freqs = np.exp(np.arange(half, dtype=np.float32) * (-math.log(10000.0) / (half - 1)))
    ang = inp["t"][:, None] * freqs[None, :]
    temb = np.concatenate([np.sin(ang), np.cos(ang)], -1)
    temb = _silu(temb @ inp["W_t1"] + inp["b_t1"])
    temb = _silu(temb @ inp["W_t2"] + inp["b_t2"])        # (B,H)
    tb = np.stack([temb @ inp["Wt"][i] + inp["bt"][i] for i in range(NB)])  # (NB,B,H)

    shared = {
        "fre": np.ascontiguousarray(fre),
        "fim": np.ascontiguousarray(fim),
        "gt": np.ascontiguousarray(gt),
        "khat": khat.astype(BF16),
        "dvec": np.ascontiguousarray(  # (NB, BH) D' per zT column
            np.tile(dfold[:, None, :], (1, BLOC, 1)).reshape(NB, BH)),
        "wo": np.ascontiguousarray(inp["Wo_s4"].astype(BF16)),   # (NB,H,H) lhsT
        "w1": np.ascontiguousarray(inp["W1"].astype(BF16)),
        "w2": np.ascontiguousarray(inp["W2"].astype(BF16)),
        "wf": np.ascontiguousarray(inp["Wf"].astype(BF16)),      # (NB,4,H)
        "wh1": np.ascontiguousarray(inp["Wh1"].astype(np.float32)),
        "wh2": np.ascontiguousarray(inp["Wh2"].astype(np.float32)),  # (H,1)
        "winv": np.ascontiguousarray(inp["W_in"].astype(np.float32)),  # (1,H)
    }
    per_core = []
    for c in range(NCORES):
        b0 = c * BLOC
        xin = np.ascontiguousarray(inp["input"][b0:b0 + BLOC, :, 0].astype(np.float32))  # (2,1024)
        featT = np.ascontiguousarray(
            np.swapaxes(inp["features"][b0:b0 + BLOC], 1, 2).astype(BF16))  # (2,4,1024)
        tbv = np.empty((P, NB * BLOC * HT), np.float32)  # col = i*4 + b*2 + ht
        for i in range(NB):
            for b in range(BLOC):
                for ht in range(HT):
                    tbv[:, i * 4 + b * 2 + ht] = tb[i, b0 + b, ht * P:(ht + 1) * P]
        per_core.append({"xin": xin, "featT": featT, "tbv": tbv})
    return shared, per_core, tb


# ---------------------------------------------------------------------------
# bass program
# ---------------------------------------------------------------------------

def _build_nc():
    global _BUILT
    if _BUILT is not None:
        return _BUILT
    import concourse.bass as bass
    import concourse.bacc as bacc
    import concourse.mybir as mybir
    import concourse.tile as tile
    from concourse.masks import make_identity

    f32 = mybir.dt.float32
    bf16 = mybir.dt.bfloat16
    AF = mybir.ActivationFunctionType
    OP = mybir.AluOpType

    nc = bacc.Bacc()

    # DRAM I/O
    d_fre = nc.dram_tensor("fre", [L, L], bf16, kind="ExternalInput")
    d_fim = nc.dram_tensor("fim", [L, L], bf16, kind="ExternalInput")
    d_gt = nc.dram_tensor("gt", [2 * L, L], bf16, kind="ExternalInput")
    d_khat = nc.dram_tensor("khat", [NB, NBT // 2, 2, P, H], bf16, kind="ExternalInput")
    d_dvec = nc.dram_tensor("dvec", [NB, BH], f32, kind="ExternalInput")
    d_wo = nc.dram_tensor("wo", [NB, H, H], bf16, kind="ExternalInput")
    d_w1 = nc.dram_tensor("w1", [NB, H, H], bf16, kind="ExternalInput")
    d_w2 = nc.dram_tensor("w2", [NB, H, H], bf16, kind="ExternalInput")
    d_wf = nc.dram_tensor("wf", [NB, NFEAT, H], bf16, kind="ExternalInput")
    d_wh1 = nc.dram_tensor("wh1", [H, H], f32, kind="ExternalInput")
    d_wh2 = nc.dram_tensor("wh2", [H, 1], f32, kind="ExternalInput")
    d_winv = nc.dram_tensor("winv", [1, H], f32, kind="ExternalInput")
    d_xin = nc.dram_tensor("xin", [BLOC, L], f32, kind="ExternalInput")
    d_featT = nc.dram_tensor("featT", [BLOC, NFEAT, L], bf16, kind="ExternalInput")
    d_tbv = nc.dram_tensor("tbv", [P, NB * BLOC * HT], f32, kind="ExternalInput")
    d_out = nc.dram_tensor("out", [BLOC, L], f32, kind="ExternalOutput")

    with tile.TileContext(nc) as tc:
        with (
            tc.tile_pool(name="mats", bufs=1) as mats,
            tc.tile_pool(name="stream", bufs=1) as stream,
            tc.tile_pool(name="ublk", bufs=1) as ublk,
            tc.tile_pool(name="zx", bufs=2) as zxp,
            tc.tile_pool(name="dg", bufs=2) as dgp,
            tc.tile_pool(name="khp", bufs=4) as khp,
            tc.tile_pool(name="yp", bufs=1) as yp,
            tc.tile_pool(name="ycp", bufs=1) as ycp,
            tc.tile_pool(name="tmp", bufs=3) as tmpp,
            tc.tile_pool(name="wts", bufs=2) as wts,
            tc.tile_pool(name="small", bufs=4) as small,
            tc.tile_pool(name="psA", bufs=4, space="PSUM") as psA,   # fwd dft Z
            tc.tile_pool(name="psB", bufs=2, space="PSUM") as psB,   # idft / transposes
            tc.tile_pool(name="psC", bufs=1, space="PSUM") as psC,   # channel matmuls
            tc.tile_pool(name="psD", bufs=1, space="PSUM") as psD,   # transposes stage B/E
        ):
            # ---- persistent loads ----
            fre_s = mats.tile([P, LT, L], bf16)
            fim_s = mats.tile([P, LT, L], bf16)
            gt_s = mats.tile([P, NBT, L], bf16)
            for lt in range(LT):
                nc.sync.dma_start(out=fre_s[:, lt, :], in_=d_fre.ap()[lt * P:(lt + 1) * P, :])
                nc.sync.dma_start(out=fim_s[:, lt, :], in_=d_fim.ap()[lt * P:(lt + 1) * P, :])
            for kt in range(NBT):
                nc.sync.dma_start(out=gt_s[:, kt, :], in_=d_gt.ap()[kt * P:(kt + 1) * P, :])

            ident = mats.tile([P, P], f32)
            make_identity(nc, ident)
            identb = mats.tile([P, P], bf16)
            make_identity(nc, identb)
            eps_s = mats.tile([P, 1], f32)
            nc.vector.memset(eps_s, 1e-5)
            tbv_s = mats.tile([P, NB * BLOC * HT], f32)
            nc.sync.dma_start(out=tbv_s, in_=d_tbv.ap())
            xin_s = mats.tile([1, BLOC, L], f32)
            nc.sync.dma_start(out=xin_s[0:1, :, :], in_=d_xin.ap().rearrange("(o b) l -> o b l", o=1))
            featT_s = mats.tile([NFEAT, BLOC, L], bf16)
            for b in range(BLOC):
                nc.sync.dma_start(out=featT_s[:, b, :], in_=d_featT.ap()[b])
            winv_s = mats.tile([1, H], f32)
            nc.sync.dma_start(out=winv_s, in_=d_winv.ap())
            wf_s = mats.tile([NFEAT, NB, H], bf16)
            for i in range(NB):
                nc.sync.dma_start(out=wf_s[:, i, :], in_=d_wf.ap()[i])

            x_s = stream.tile([P, BLOC * HT, L], bf16)
            skip_s = stream.tile([P, BLOC * HT, L], f32)
            nc.vector.memset(skip_s, 0.0)

            # ---- input projection: x = relu(input @ W_in) ----
            for b in range(BLOC):
                for ht in range(HT):
                    for nch in range(2):
                        pin = psC.tile([P, 512], f32, tag="ps")
                        nc.tensor.matmul(
                            pin,
                            winv_s[0:1, ht * P:(ht + 1) * P],
                            xin_s[0:1, b, nch * 512:(nch + 1) * 512],
                            start=True, stop=True)
                        nc.scalar.activation(
                            x_s[:, b * HT + ht, nch * 512:(nch + 1) * 512], pin, AF.Relu)

            # ---- blocks ----
            for i in range(NB):
                # per-block weight/kernel loads
                wo_s = wts.tile([P, HT, H], bf16, tag="wo")
                w1_s = wts.tile([P, HT, H], bf16, tag="w1")
                w2_s = wts.tile([P, HT, H], bf16, tag="w2")
                for kt in range(HT):
                    nc.sync.dma_start(out=wo_s[:, kt, :], in_=d_wo.ap()[i, kt * P:(kt + 1) * P, :])
                    nc.sync.dma_start(out=w1_s[:, kt, :], in_=d_w1.ap()[i, kt * P:(kt + 1) * P, :])
                    nc.sync.dma_start(out=w2_s[:, kt, :], in_=d_w2.ap()[i, kt * P:(kt + 1) * P, :])
                dexp_s = wts.tile([P, BH], f32, tag="dexp")
                dv = d_dvec.ap()[i]
                nc.sync.dma_start(
                    out=dexp_s,
                    in_=bass.AP(tensor=dv.tensor, offset=dv.offset, ap=[[0, P]] + list(dv.ap)))

                # A: u = x + tb  (xH, f32)
                u_s = ublk.tile([P, BLOC * HT, L], f32, tag="u")
                for j in range(BLOC * HT):
                    nc.scalar.activation(
                        u_s[:, j, :], x_s[:, j, :], AF.Identity,
                        bias=tbv_s[:, i * 4 + j:i * 4 + j + 1], scale=1.0)

                # B: transpose u -> zT; layernorm -> z (bf16); dz = z*D'
                zbf_s = zxp.tile([P, LT, BH], bf16, tag="zx")
                dz_s = dgp.tile([P, LT, BH], bf16, tag="dg")
                for lt in range(LT):
                    pt = psD.tile([P, BH], f32, tag="t")
                    for j in range(BLOC * HT):
                        nc.tensor.transpose(
                            pt[:, j * P:(j + 1) * P],
                            u_s[:, j, lt * P:(lt + 1) * P], ident)
                    st = small.tile([P, BLOC, 6], f32, tag="st")
                    mv = small.tile([P, BLOC, 2], f32, tag="mv")
                    rs = small.tile([P, BLOC], f32, tag="rs")
                    for b in range(BLOC):
                        nc.vector.bn_stats(st[:, b, :], pt[:, b * H:(b + 1) * H])
                        nc.vector.bn_aggr(mv[:, b, :], st[:, b, :])
                        nc.scalar.activation(rs[:, b:b + 1], mv[:, b, 1:2], AF.Sqrt,
                                             bias=eps_s, scale=1.0)
                        nc.vector.reciprocal(rs[:, b:b + 1], rs[:, b:b + 1])
                        nc.vector.tensor_scalar(
                            out=zbf_s[:, lt, b * H:(b + 1) * H],
                            in0=pt[:, b * H:(b + 1) * H],
                            scalar1=mv[:, b, 0:1], scalar2=rs[:, b:b + 1],
                            op0=OP.subtract, op1=OP.mult)
                    nc.gpsimd.tensor_mul(dz_s[:, lt, :], zbf_s[:, lt, :], dexp_s)

                # C: fwd DFT + pointwise multiply by Khat
                y_s = yp.tile([P, NBT, BH], bf16)
                for mt in range(NBT // 2):
                    kh = khp.tile([P, 2, BH], bf16, tag="kh")
                    ksrc = d_khat.ap()[i, mt]  # (2, P, H)
                    for ri in range(2):
                        s = ksrc[ri]
                        nc.sync.dma_start(
                            out=kh[:, ri, :],
                            in_=bass.AP(tensor=s.tensor, offset=s.offset,
                                        ap=[list(s.ap[0]), [0, BLOC], list(s.ap[1])]))
                    zre = psA.tile([P, BH], f32, tag="z")
                    zim = psA.tile([P, BH], f32, tag="z")
                    for lt in range(LT):
                        nc.tensor.matmul(zre, fre_s[:, lt, mt * P:(mt + 1) * P],
                                         zbf_s[:, lt, :], start=(lt == 0), stop=(lt == LT - 1))
                    for lt in range(LT):
                        nc.tensor.matmul(zim, fim_s[:, lt, mt * P:(mt + 1) * P],
                                         zbf_s[:, lt, :], start=(lt == 0), stop=(lt == LT - 1))
                    ta = tmpp.tile([P, BH], f32, tag="tmp")
                    tb_ = tmpp.tile([P, BH], f32, tag="tmp")
                    nc.vector.tensor_mul(ta, zre, kh[:, 0, :])
                    nc.vector.tensor_mul(tb_, zim, kh[:, 1, :])
                    nc.vector.tensor_sub(y_s[:, mt, :], ta, tb_)
                    tc_ = tmpp.tile([P, BH], f32, tag="tmp")
                    td = tmpp.tile([P, BH], f32, tag="tmp")
                    nc.vector.tensor_mul(tc_, zre, kh[:, 1, :])
                    nc.vector.tensor_mul(td, zim, kh[:, 0, :])
                    nc.vector.tensor_add(y_s[:, mt + NBT // 2, :], tc_, td)

                # D: inverse DFT + +dz + gelu -> yc (zT bf16)
                yc_s = ycp.tile([P, LT, BH], bf16, tag="yc")
                for tt in range(LT):
                    py = psB.tile([P, BH], f32, tag="y")
                    for kt in range(NBT):
                        nc.tensor.matmul(py, gt_s[:, kt, tt * P:(tt + 1) * P],
                                         y_s[:, kt, :], start=(kt == 0), stop=(kt == NBT - 1))
                    tg = tmpp.tile([P, BH], f32, tag="tmp")
                    nc.vector.tensor_add(tg, py, dz_s[:, tt, :])
                    nc.scalar.activation(yc_s[:, tt, :], tg, AF.Gelu)

                # E: transpose yc -> yx (xH bf16)
                yx_s = zxp.tile([P, BLOC * HT, L], bf16, tag="zx")
                for j in range(BLOC * HT):
                    for nch in range(2):
                        pt2 = psD.tile([P, BH], bf16, tag="t")
                        for q in range(4):
                            lt = nch * 4 + q
                            nc.tensor.transpose(
                                pt2[:, q * P:(q + 1) * P],
                                yc_s[:, lt, j * P:(j + 1) * P], identb)
                        nc.scalar.copy(yx_s[:, j, nch * 512:(nch + 1) * 512], pt2)

                # F: out = Wo^T yx + Wf^T feat + u ; g = tanh(out)*sigmoid(out)
                g_s = dgp.tile([P, BLOC * HT, L], bf16, tag="dg")
                for b in range(BLOC):
                    for ot in range(HT):
                        for nch in range(2):
                            po = psC.tile([P, 512], f32, tag="ps")
                            for kt in range(HT):
                                nc.tensor.matmul(
                                    po, wo_s[:, kt, ot * P:(ot + 1) * P],
                                    yx_s[:, b * HT + kt, nch * 512:(nch + 1) * 512],
                                    start=(kt == 0), stop=False)
                            nc.tensor.matmul(
                                po, wf_s[:, i, ot * P:(ot + 1) * P],
                                featT_s[:, b, nch * 512:(nch + 1) * 512],
                                start=False, stop=True)
                            j = b * HT + ot
                            sl = slice(nch * 512, (nch + 1) * 512)
                            t2 = tmpp.tile([P, 512], f32, tag="tmp")
                            nc.vector.tensor_add(t2, po, u_s[:, j, sl])
                            th = tmpp.tile([P, 512], f32, tag="tmp")
                            sg = tmpp.tile([P, 512], f32, tag="tmp")
                            nc.scalar.activation(th, t2, AF.Tanh)
                            nc.scalar.activation(sg, t2, AF.Sigmoid)
                            nc.gpsimd.tensor_mul(g_s[:, j, sl], th, sg)

                # G: x += W1^T g ; skip += W2^T g
                for b in range(BLOC):
                    for ot in range(HT):
                        for nch in range(2):
                            j = b * HT + ot
                            sl = slice(nch * 512, (nch + 1) * 512)
                            p1 = psC.tile([P, 512], f32, tag="ps")
                            for kt in range(HT):
                                nc.tensor.matmul(
                                    p1, w1_s[:, kt, ot * P:(ot + 1) * P],
                                    g_s[:, b * HT + kt, sl],
                                    start=(kt == 0), stop=(kt == HT - 1))
                            nc.vector.tensor_add(x_s[:, j, sl], p1, x_s[:, j, sl])
                            p2 = psC.tile([P, 512], f32, tag="ps")
                            for kt in range(HT):
                                nc.tensor.matmul(
                                    p2, w2_s[:, kt, ot * P:(ot + 1) * P],
                                    g_s[:, b * HT + kt, sl],
                                    start=(kt == 0), stop=(kt == HT - 1))
                            nc.vector.tensor_add(skip_s[:, j, sl], p2, skip_s[:, j, sl])

            # ---- head: out = relu(skip^T Wh1) Wh2 + input ----
            wh1_s = mats.tile([P, HT, H], f32)
            for kt in range(HT):
                nc.sync.dma_start(out=wh1_s[:, kt, :], in_=d_wh1.ap()[kt * P:(kt + 1) * P, :])
            wh2_s = mats.tile([P, HT, 1], f32)
            for kt in range(HT):
                nc.sync.dma_start(out=wh2_s[:, kt, :], in_=d_wh2.ap()[kt * P:(kt + 1) * P, :])
            h1_s = ublk.tile([P, BLOC * HT, L], f32, tag="u")
            for b in range(BLOC):
                for ot in range(HT):
                    for nch in range(2):
                        ph = psC.tile([P, 512], f32, tag="ps")
                        for kt in range(HT):
                            nc.tensor.matmul(
                                ph, wh1_s[:, kt, ot * P:(ot + 1) * P],
                                skip_s[:, b * HT + kt, nch * 512:(nch + 1) * 512],
                                start=(kt == 0), stop=(kt == HT - 1))
                        nc.scalar.activation(
                            h1_s[:, b * HT + ot, nch * 512:(nch + 1) * 512], ph, AF.Relu)
            o_s = ycp.tile([1, BLOC, L], f32, tag="yc")
            for b in range(BLOC):
                for nch in range(2):
                    ph2 = psC.tile([1, 512], f32, tag="ps")
                    for kt in range(HT):
                        nc.tensor.matmul(
                            ph2, wh2_s[:, kt, :],
                            h1_s[:, b * HT + kt, nch * 512:(nch + 1) * 512],
                            start=(kt == 0), stop=(kt == HT - 1))
                    nc.vector.tensor_add(
                        o_s[0:1, b, nch * 512:(nch + 1) * 512], ph2,
                        xin_s[0:1, b, nch * 512:(nch + 1) * 512])
            nc.sync.dma_start(out=d_out.ap().rearrange("(o b) l -> o b l", o=1), in_=o_s[0:1, :, :])

    nc.finalize()
    _BUILT = nc
    return nc


# ---------------------------------------------------------------------------
# entry points
# ---------------------------------------------------------------------------

def _in_maps(shared, per_core):
    maps = []
    for c in range(NCORES):
        m = dict(shared)
        m.update(per_core[c])
        maps.append(m)
    return maps


def kernel(**inputs):
    global _LAST_EXEC_NS
    inp = {k: np.asarray(v) for k, v in inputs.items()}
    shared, per_core, _ = _host_prep(inp)
    nc = _build_nc()
    from concourse.bass_utils import run_bass_kernel_spmd
    import os
    trace = bool(os.environ.get("K_TRACE"))
    r = run_bass_kernel_spmd(nc, _in_maps(shared, per_core),
                             core_ids=list(range(NCORES)), trace=trace)
    _LAST_EXEC_NS = r.exec_time_ns
    out = np.stack([r.results[c]["out"] for c in range(NCORES)])  # (8,2,1024)
    return out.reshape(B, L, 1).astype(np.float32)


def _run_sim(inputs, core=0):
    """CoreSim single-core check (dev only)."""
    inp = {k: np.asarray(v) for k, v in inputs.items()}
    shared, per_core, _ = _host_prep(inp)
    nc = _build_nc()
    from concourse.bass_interp import CoreSim
    sim = CoreSim(nc)
    m = dict(shared); m.update(per_core[core])
    for name, val in m.items():
        sim.tensor(name)[:] = val
    sim.simulate(check_with_hw=False)
    return np.array(sim.tensor("out"))
